# revision 12
# baseline (speedup 1.0000x reference)
"""Trainium2 Bass kernel for nn_BitAllocationNetwork.

Strategy (pure data parallel over batch, 8 cores):
  - The only heavy compute is reducing importance_scores [16, 2048, 4096]
    over T=2048 (512 MiB streamed from HBM) -> memory-bound; per core
    64 MiB at ~358 GB/s => ~187 us roofline.
  - Each core streams its [2, 2048, 4096] shard as 32 tiles of
    [128 T-rows, 4096]. Tiles are reduced by three engine lanes in
    parallel so no engine comes close to the DMA roofline:
      'pe'  (TensorE): ones-indicator matmul (fp32) accumulates per-batch
            column sums over T into PSUM ([2, 512] x 8 banks). A [128, 2]
            indicator lhsT (column b = 1 for batch b) lets both local
            batches share one accumulation chain (matmul PSUM outputs must
            start at partition 0/32/64/96, so per-batch partition offsets
            are not an option).
      'act' (ScalarE): 8 chunked Copy activations per tile with
            accum_out -> per-partition per-group sums into this tile's own
            8 accumulator columns (no cross-tile dependencies).
      'dve' (VectorE): one tensor_reduce over a [128, 8, 512] view ->
            [128, 8] per-partition group sums into this tile's own
            8 accumulator columns.
  - Device outputs per core: colsum_pe [2, 4096] (pe-lane partial) and
    acc [128, 8 * n_slots] (act/dve-lane partials). Host combines into
    group sums, then runs the tiny MLP chain ([16, 8] tensors, ~10^4
    flops) in float64.
"""

import numpy as np

import concourse.mybir as mybir
import concourse.tile as tile
from concourse import bacc
from concourse.bass_utils import run_bass_kernel_spmd

# Problem shapes (hardcoded per harness contract)
B, T, D = 16, 2048, 4096
G, GS = 8, 512
N_CORES = 8
BL = B // N_CORES  # batches per core
TP = 128  # T rows per SBUF tile (partition dim)
NT = T // TP  # T tiles per batch
NJ = D // 512  # 512-wide slices per tile

MIN_BITS, MAX_BITS, TARGET_BITS = 2.0, 8.0, 4.0
BIT_LEVELS = np.array([2.0, 4.0, 8.0], dtype=np.float64)
LN_EPS = 1e-5


def _default_lanes():
    # HW-tuned split: VectorE reduces 3 of every 4 tiles, ScalarE the rest.
    # (TensorE matmul lane measured slower overall: fp32 matmul is 4
    # cycles/row and its accumulation chain lengthens the kernel tail.)
    return ["act" if k % 4 == 3 else "dve" for k in range(BL * NT)]


LANES = _default_lanes()
TILES_PER_LOAD = 1  # T-tiles fetched per dma_start (1, 2, or 4)
DUAL_RING = False  # alternate DMA issue between sync (SP) and scalar (ACT) HWDGE


def set_lanes(name):
    """Switch the lane pattern (clears the build cache). Bench/tuning only."""
    global LANES, TILES_PER_LOAD, DUAL_RING
    N = BL * NT
    if name.endswith("_dual"):
        DUAL_RING = True
        name = name[: -len("_dual")]
    else:
        DUAL_RING = False
    if name[-1] in "24":
        TILES_PER_LOAD = int(name[-1])
        name = name[:-1]
    else:
        TILES_PER_LOAD = 1
    if name == "pe3act":
        LANES = ["pe" if k % 3 == 0 else "act" for k in range(N)]
    elif name == "actdve":
        LANES = ["act" if k % 2 == 0 else "dve" for k in range(N)]
    elif name == "alldve":
        LANES = ["dve"] * N
    elif name == "allact":
        LANES = ["act"] * N
    elif name == "3lane":
        LANES = _default_lanes()
    elif name == "dmaonly":
        LANES = ["tiny"] * N
    elif name == "dve3act1":
        LANES = ["act" if k % 4 == 3 else "dve" for k in range(N)]
    else:
        raise ValueError(name)
    _NC_CACHE.clear()


_NC_CACHE = {}


def _acc_layout(lanes):
    """(local_batch, col, width) per non-pe tile in stream order (one rep)."""
    out = []
    col = 0
    for k, lane in enumerate(lanes):
        if lane == "pe":
            continue
        width = 1 if lane == "tiny" else G
        out.append((k // NT, col, width))
        col += width
    return out, col


def _build_bass(reps=1, lanes=None):
    """Per-core kernel: x [BL*T, D] f32 -> colsum_pe [BL, D], acc [128, 8*n].

    reps > 1 repeats the full streaming pass (bench-only, for on-device
    timing): one PSUM accumulation chain spans all reps and each rep's
    act/dve tiles write distinct accumulator columns, so no rep's work is
    dead.
    """
    if lanes is None:
        lanes = LANES
    layout, acc_w = _acc_layout(lanes)

    nc = bacc.Bacc("TRN2", target_bir_lowering=False)
    x = nc.dram_tensor("x", [BL * T, D], mybir.dt.float32, kind="ExternalInput")
    out_pe = nc.dram_tensor(
        "colsum_pe", [BL, D], mybir.dt.float32, kind="ExternalOutput"
    )
    out_acc = nc.dram_tensor(
        "acc", [128, acc_w * reps], mybir.dt.float32, kind="ExternalOutput"
    )

    pe_ks = [k for k, lane in enumerate(lanes) if lane == "pe"]
    first_pe = pe_ks[0] if pe_ks else None
    last_pe = pe_ks[-1] if pe_ks else None

    with tile.TileContext(nc) as tc:
        with (
            tc.tile_pool(name="const", bufs=1) as cpool,
            tc.tile_pool(name="xin", bufs=(4 if TILES_PER_LOAD == 1 else 3)) as xpool,
            tc.tile_pool(name="scratch", bufs=3) as spool,
            tc.tile_pool(name="accs", bufs=1) as apool,
            tc.tile_pool(name="res", bufs=1) as rpool,
            tc.tile_pool(name="ps", bufs=1, space="PSUM") as ppool,
        ):
            # sel[:, BL*b : BL*(b+1)] is lhsT for batch b: column b ones.
            sel = cpool.tile([128, BL * BL], mybir.dt.float32)
            nc.vector.memset(sel[:, :], 0.0)
            for b in range(BL):
                nc.vector.memset(sel[:, BL * b + b : BL * b + b + 1], 1.0)

            acc = apool.tile([128, acc_w * reps], mybir.dt.float32)

            psums = [
                ppool.tile([BL, 512], mybir.dt.float32, tag=f"ps{j}", name=f"ps{j}")
                for j in range(NJ)
            ]

            slot_cols = [col for (_b, col, _w) in layout]
            tpl = TILES_PER_LOAD
            assert NT % tpl == 0
            for rep in range(reps):
                slot = 0
                for b in range(BL):
                    for tc0 in range(NT // tpl):
                        ks = [b * NT + tc0 * tpl + i for i in range(tpl)]
                        xt = xpool.tile(
                            [TP, tpl, D], mybir.dt.float32, tag="xt", name="xt"
                        )
                        r0 = b * T + tc0 * tpl * TP
                        src = x[r0 : r0 + tpl * TP, :].rearrange(
                            "(c p) d -> p c d", c=tpl
                        )
                        eng = (
                            nc.scalar
                            if (DUAL_RING and (tc0 + b * (NT // tpl)) % 2)
                            else nc.sync
                        )
                        eng.dma_start(out=xt[:, :, :], in_=src)
                        for i, k in enumerate(ks):
                            lane = lanes[k]
                            if lane == "pe":
                                lhsT = sel[:, BL * b : BL * (b + 1)]
                                for j in range(NJ):
                                    nc.tensor.matmul(
                                        psums[j][:, :],
                                        lhsT,
                                        xt[:, i, j * 512 : (j + 1) * 512],
                                        start=(rep == 0 and k == first_pe),
                                        stop=(rep == reps - 1 and k == last_pe),
                                    )
                            elif lane == "act":
                                scratch = spool.tile(
                                    [TP, 512], mybir.dt.float32, tag="scr", name="scr"
                                )
                                for j in range(NJ):
                                    c = rep * acc_w + slot_cols[slot] + j
                                    nc.scalar.activation(
                                        scratch[:, :],
                                        xt[:, i, j * 512 : (j + 1) * 512],
                                        mybir.ActivationFunctionType.Copy,
                                        accum_out=acc[:, c : c + 1],
                                    )
                                slot += 1
                            elif lane == "dve":
                                c = rep * acc_w + slot_cols[slot]
                                nc.vector.reduce_sum(
                                    acc[:, c : c + G],
                                    xt[:, i, :].rearrange("p (g s) -> p g s", g=G),
                                    axis=mybir.AxisListType.X,
                                )
                                slot += 1
                            else:  # tiny: keep the load live, ~zero compute
                                c = rep * acc_w + slot_cols[slot]
                                nc.vector.reduce_sum(
                                    acc[:, c : c + 1], xt[:, i, 0:8],
                                    axis=mybir.AxisListType.X,
                                )
                                slot += 1

            res = rpool.tile([BL, D], mybir.dt.float32)
            if pe_ks:
                for j in range(NJ):
                    nc.vector.tensor_copy(
                        res[:, j * 512 : (j + 1) * 512], psums[j][:, :]
                    )
            else:
                nc.vector.memset(res[:, :], 0.0)
            nc.sync.dma_start(out=out_pe[:, :], in_=res[:, :])
            nc.sync.dma_start(out=out_acc[:, :], in_=acc[:, :])

    nc.finalize()
    return nc


def _get_nc(reps=1):
    if reps not in _NC_CACHE:
        _NC_CACHE[reps] = _build_bass(reps)
    return _NC_CACHE[reps]


def _erf(x):
    from math import erf

    return np.vectorize(erf)(x)


def _host_tail(group_imp, u, W1, b1, ln_g, ln_b, W2, b2, bit_embeddings):
    """Tiny MLP chain + allocation on [B, G] tensors, float64."""
    h = group_imp @ W1 + b1
    h = 0.5 * h * (1.0 + _erf(h / np.sqrt(2.0)))  # exact gelu
    mu = h.mean(axis=-1, keepdims=True)
    var = ((h - mu) ** 2).mean(axis=-1, keepdims=True)
    h = (h - mu) / np.sqrt(var + LN_EPS) * ln_g + ln_b
    logits = h @ W2 + b2

    gumbel = -np.log(-np.log(u + 1e-8) + 1e-8)
    z = logits + gumbel
    z = z - z.max(axis=-1, keepdims=True)
    ez = np.exp(z)
    bit_probs = ez / ez.sum(axis=-1, keepdims=True)

    alloc = MIN_BITS + bit_probs * (MAX_BITS - MIN_BITS)
    budget = TARGET_BITS * G
    alloc = alloc * (budget / alloc.sum(axis=-1, keepdims=True))
    alloc = np.clip(alloc, MIN_BITS, MAX_BITS)

    dist = np.abs(alloc[..., None] - BIT_LEVELS)
    idx = np.argmin(dist, axis=-1)
    discrete_bits = BIT_LEVELS[idx]

    y = np.exp(-np.abs(discrete_bits[..., None] - BIT_LEVELS))
    ey = np.exp(y - y.max(axis=-1, keepdims=True))
    bit_w = ey / ey.sum(axis=-1, keepdims=True)
    embeddings = np.einsum("bgl,ld->bgd", bit_w, bit_embeddings)

    group_indices = np.broadcast_to(
        (np.arange(D, dtype=np.int32) // GS)[None, :], (B, D)
    ).astype(np.int32)
    bitrate_loss = (discrete_bits.mean() - TARGET_BITS) ** 2
    diversity_bonus = -(bit_probs * np.log(bit_probs + 1e-8)).sum(axis=-1).mean()

    return (
        discrete_bits.astype(np.float32),
        group_indices,
        embeddings.astype(np.float32),
        np.float32(bitrate_loss),
        np.float32(diversity_bonus),
        bit_probs.astype(np.float32),
    )


def _run_device(importance_scores, **run_kwargs):
    """Run the Bass kernel on all 8 cores; returns ([B, G] group sums, results)."""
    nc = _get_nc()
    in_maps = [
        {"x": importance_scores[i * BL : (i + 1) * BL].reshape(BL * T, D)}
        for i in range(N_CORES)
    ]
    r = run_bass_kernel_spmd(nc, in_maps, core_ids=list(range(N_CORES)), **run_kwargs)

    layout, acc_w = _acc_layout(LANES)

    group_sums = np.zeros((B, G), dtype=np.float64)
    for i, res in enumerate(r.results):
        pe = res["colsum_pe"].astype(np.float64)  # [BL, D]
        group_sums[i * BL : (i + 1) * BL] += pe.reshape(BL, G, GS).sum(axis=-1)
        acc = res["acc"].astype(np.float64).sum(axis=0)  # [acc_w]
        for b, col, width in layout:
            assert width == G, "tiny lane is bench-only"
            group_sums[i * BL + b] += acc[col : col + width]
    return group_sums, r


def kernel(importance_scores, u, W1, b1, ln_g, ln_b, W2, b2, bit_embeddings):
    importance_scores = np.ascontiguousarray(
        np.asarray(importance_scores, dtype=np.float32)
    )

    group_sums, _ = _run_device(importance_scores)
    group_imp = group_sums / (T * GS)

    return _host_tail(
        group_imp,
        np.asarray(u, dtype=np.float64),
        np.asarray(W1, dtype=np.float64),
        np.asarray(b1, dtype=np.float64),
        np.asarray(ln_g, dtype=np.float64),
        np.asarray(ln_b, dtype=np.float64),
        np.asarray(W2, dtype=np.float64),
        np.asarray(b2, dtype=np.float64),
        np.asarray(bit_embeddings, dtype=np.float64),
    )


# revision 14
# speedup vs baseline: 1.7148x; 1.7148x over previous
"""Trainium2 Bass kernel for nn_BitAllocationNetwork.

Strategy (pure data parallel over batch, 8 cores):
  - The only heavy compute is reducing importance_scores [16, 2048, 4096]
    over T=2048 (512 MiB streamed from HBM) -> memory-bound; per core
    64 MiB at ~358 GB/s => ~187 us roofline.
  - Each core streams its [2, 2048, 4096] shard as 32 tiles of
    [128 T-rows, 4096]. Tiles are reduced by three engine lanes in
    parallel so no engine comes close to the DMA roofline:
      'pe'  (TensorE): ones-indicator matmul (fp32) accumulates per-batch
            column sums over T into PSUM ([2, 512] x 8 banks). A [128, 2]
            indicator lhsT (column b = 1 for batch b) lets both local
            batches share one accumulation chain (matmul PSUM outputs must
            start at partition 0/32/64/96, so per-batch partition offsets
            are not an option).
      'act' (ScalarE): 8 chunked Copy activations per tile with
            accum_out -> per-partition per-group sums into this tile's own
            8 accumulator columns (no cross-tile dependencies).
      'dve' (VectorE): one tensor_reduce over a [128, 8, 512] view ->
            [128, 8] per-partition group sums into this tile's own
            8 accumulator columns.
  - Device outputs per core: colsum_pe [2, 4096] (pe-lane partial) and
    acc [128, 8 * n_slots] (act/dve-lane partials). Host combines into
    group sums, then runs the tiny MLP chain ([16, 8] tensors, ~10^4
    flops) in float64.
"""

import numpy as np

import concourse.mybir as mybir
import concourse.tile as tile
from concourse import bacc
from concourse.bass_utils import run_bass_kernel_spmd

# Problem shapes (hardcoded per harness contract)
B, T, D = 16, 2048, 4096
G, GS = 8, 512
N_CORES = 8
BL = B // N_CORES  # batches per core
TP = 128  # T rows per SBUF tile (partition dim)
NT = T // TP  # T tiles per batch
NJ = D // 512  # 512-wide slices per tile

MIN_BITS, MAX_BITS, TARGET_BITS = 2.0, 8.0, 4.0
BIT_LEVELS = np.array([2.0, 4.0, 8.0], dtype=np.float64)
LN_EPS = 1e-5


def _default_lanes():
    # HW-tuned split: VectorE reduces 3 of every 4 tiles, ScalarE the rest.
    # (TensorE matmul lane measured slower overall: fp32 matmul is 4
    # cycles/row and its accumulation chain lengthens the kernel tail.)
    return ["act" if k % 4 == 3 else "dve" for k in range(BL * NT)]


LANES = _default_lanes()
TILES_PER_LOAD = 1  # T-tiles fetched per dma_start (1, 2, or 4)
DUAL_RING = False  # alternate DMA issue between sync (SP) and scalar (ACT) HWDGE
XIN_BUFS = 4  # input tile double-buffer depth


def set_lanes(name):
    """Switch the lane pattern (clears the build cache). Bench/tuning only."""
    global LANES, TILES_PER_LOAD, DUAL_RING, XIN_BUFS
    N = BL * NT
    if name.endswith("_dual"):
        DUAL_RING = True
        name = name[: -len("_dual")]
    else:
        DUAL_RING = False
    if "_b" in name:
        name, bs = name.split("_b")
        XIN_BUFS = int(bs)
    else:
        XIN_BUFS = 4
    if name[-1] in "24":
        TILES_PER_LOAD = int(name[-1])
        name = name[:-1]
    else:
        TILES_PER_LOAD = 1
    if name == "pe3act":
        LANES = ["pe" if k % 3 == 0 else "act" for k in range(N)]
    elif name == "actdve":
        LANES = ["act" if k % 2 == 0 else "dve" for k in range(N)]
    elif name == "alldve":
        LANES = ["dve"] * N
    elif name == "allact":
        LANES = ["act"] * N
    elif name == "3lane":
        LANES = _default_lanes()
    elif name == "dmaonly":
        LANES = ["tiny"] * N
    elif name == "dve3act1":
        LANES = ["act" if k % 4 == 3 else "dve" for k in range(N)]
    else:
        raise ValueError(name)
    _NC_CACHE.clear()


_NC_CACHE = {}


def _acc_layout(lanes):
    """(local_batch, col, width) per non-pe tile in stream order (one rep)."""
    out = []
    col = 0
    for k, lane in enumerate(lanes):
        if lane == "pe":
            continue
        width = 1 if lane == "tiny" else G
        out.append((k // NT, col, width))
        col += width
    return out, col


def _build_bass(reps=1, lanes=None):
    """Per-core kernel: x [BL*T, D] f32 -> colsum_pe [BL, D], acc [128, 8*n].

    reps > 1 repeats the full streaming pass (bench-only, for on-device
    timing): one PSUM accumulation chain spans all reps and each rep's
    act/dve tiles write distinct accumulator columns, so no rep's work is
    dead.
    """
    if lanes is None:
        lanes = LANES
    layout, acc_w = _acc_layout(lanes)

    nc = bacc.Bacc("TRN2", target_bir_lowering=False)
    x = nc.dram_tensor("x", [BL * T, D], mybir.dt.float32, kind="ExternalInput")
    out_pe = nc.dram_tensor(
        "colsum_pe", [BL, D], mybir.dt.float32, kind="ExternalOutput"
    )
    out_acc = nc.dram_tensor(
        "acc", [128, acc_w * reps], mybir.dt.float32, kind="ExternalOutput"
    )

    pe_ks = [k for k, lane in enumerate(lanes) if lane == "pe"]
    first_pe = pe_ks[0] if pe_ks else None
    last_pe = pe_ks[-1] if pe_ks else None

    with tile.TileContext(nc) as tc:
        with (
            tc.tile_pool(name="const", bufs=1) as cpool,
            tc.tile_pool(name="xin", bufs=(XIN_BUFS if TILES_PER_LOAD == 1 else 3)) as xpool,
            tc.tile_pool(name="scratch", bufs=3) as spool,
            tc.tile_pool(name="accs", bufs=1) as apool,
            tc.tile_pool(name="res", bufs=1) as rpool,
            tc.tile_pool(name="ps", bufs=1, space="PSUM") as ppool,
        ):
            # sel[:, BL*b : BL*(b+1)] is lhsT for batch b: column b ones.
            sel = cpool.tile([128, BL * BL], mybir.dt.float32)
            nc.vector.memset(sel[:, :], 0.0)
            for b in range(BL):
                nc.vector.memset(sel[:, BL * b + b : BL * b + b + 1], 1.0)

            acc = apool.tile([128, acc_w * reps], mybir.dt.float32)

            psums = [
                ppool.tile([BL, 512], mybir.dt.float32, tag=f"ps{j}", name=f"ps{j}")
                for j in range(NJ)
            ]

            slot_cols = [col for (_b, col, _w) in layout]
            tpl = TILES_PER_LOAD
            assert NT % tpl == 0
            for rep in range(reps):
                slot = 0
                for b in range(BL):
                    for tc0 in range(NT // tpl):
                        ks = [b * NT + tc0 * tpl + i for i in range(tpl)]
                        xt = xpool.tile(
                            [TP, tpl, D], mybir.dt.float32, tag="xt", name="xt"
                        )
                        r0 = b * T + tc0 * tpl * TP
                        src = x[r0 : r0 + tpl * TP, :].rearrange(
                            "(c p) d -> p c d", c=tpl
                        )
                        eng = (
                            nc.scalar
                            if (DUAL_RING and (tc0 + b * (NT // tpl)) % 2)
                            else nc.sync
                        )
                        eng.dma_start(out=xt[:, :, :], in_=src)
                        for i, k in enumerate(ks):
                            lane = lanes[k]
                            if lane == "pe":
                                lhsT = sel[:, BL * b : BL * (b + 1)]
                                for j in range(NJ):
                                    nc.tensor.matmul(
                                        psums[j][:, :],
                                        lhsT,
                                        xt[:, i, j * 512 : (j + 1) * 512],
                                        start=(rep == 0 and k == first_pe),
                                        stop=(rep == reps - 1 and k == last_pe),
                                    )
                            elif lane == "act":
                                scratch = spool.tile(
                                    [TP, 512], mybir.dt.float32, tag="scr", name="scr"
                                )
                                for j in range(NJ):
                                    c = rep * acc_w + slot_cols[slot] + j
                                    nc.scalar.activation(
                                        scratch[:, :],
                                        xt[:, i, j * 512 : (j + 1) * 512],
                                        mybir.ActivationFunctionType.Copy,
                                        accum_out=acc[:, c : c + 1],
                                    )
                                slot += 1
                            elif lane == "dve":
                                c = rep * acc_w + slot_cols[slot]
                                nc.vector.reduce_sum(
                                    acc[:, c : c + G],
                                    xt[:, i, :].rearrange("p (g s) -> p g s", g=G),
                                    axis=mybir.AxisListType.X,
                                )
                                slot += 1
                            else:  # tiny: keep the load live, ~zero compute
                                c = rep * acc_w + slot_cols[slot]
                                nc.vector.reduce_sum(
                                    acc[:, c : c + 1], xt[:, i, 0:8],
                                    axis=mybir.AxisListType.X,
                                )
                                slot += 1

            res = rpool.tile([BL, D], mybir.dt.float32)
            if pe_ks:
                for j in range(NJ):
                    nc.vector.tensor_copy(
                        res[:, j * 512 : (j + 1) * 512], psums[j][:, :]
                    )
            else:
                nc.vector.memset(res[:, :], 0.0)
            nc.sync.dma_start(out=out_pe[:, :], in_=res[:, :])
            nc.sync.dma_start(out=out_acc[:, :], in_=acc[:, :])

    nc.finalize()
    return nc


def _get_nc(reps=1):
    if reps not in _NC_CACHE:
        _NC_CACHE[reps] = _build_bass(reps)
    return _NC_CACHE[reps]


def _erf(x):
    from math import erf

    return np.vectorize(erf)(x)


def _host_tail(group_imp, u, W1, b1, ln_g, ln_b, W2, b2, bit_embeddings):
    """Tiny MLP chain + allocation on [B, G] tensors, float64."""
    h = group_imp @ W1 + b1
    h = 0.5 * h * (1.0 + _erf(h / np.sqrt(2.0)))  # exact gelu
    mu = h.mean(axis=-1, keepdims=True)
    var = ((h - mu) ** 2).mean(axis=-1, keepdims=True)
    h = (h - mu) / np.sqrt(var + LN_EPS) * ln_g + ln_b
    logits = h @ W2 + b2

    gumbel = -np.log(-np.log(u + 1e-8) + 1e-8)
    z = logits + gumbel
    z = z - z.max(axis=-1, keepdims=True)
    ez = np.exp(z)
    bit_probs = ez / ez.sum(axis=-1, keepdims=True)

    alloc = MIN_BITS + bit_probs * (MAX_BITS - MIN_BITS)
    budget = TARGET_BITS * G
    alloc = alloc * (budget / alloc.sum(axis=-1, keepdims=True))
    alloc = np.clip(alloc, MIN_BITS, MAX_BITS)

    dist = np.abs(alloc[..., None] - BIT_LEVELS)
    idx = np.argmin(dist, axis=-1)
    discrete_bits = BIT_LEVELS[idx]

    y = np.exp(-np.abs(discrete_bits[..., None] - BIT_LEVELS))
    ey = np.exp(y - y.max(axis=-1, keepdims=True))
    bit_w = ey / ey.sum(axis=-1, keepdims=True)
    embeddings = np.einsum("bgl,ld->bgd", bit_w, bit_embeddings)

    group_indices = np.broadcast_to(
        (np.arange(D, dtype=np.int32) // GS)[None, :], (B, D)
    ).astype(np.int32)
    bitrate_loss = (discrete_bits.mean() - TARGET_BITS) ** 2
    diversity_bonus = -(bit_probs * np.log(bit_probs + 1e-8)).sum(axis=-1).mean()

    return (
        discrete_bits.astype(np.float32),
        group_indices,
        embeddings.astype(np.float32),
        np.float32(bitrate_loss),
        np.float32(diversity_bonus),
        bit_probs.astype(np.float32),
    )


def _run_device(importance_scores, **run_kwargs):
    """Run the Bass kernel on all 8 cores; returns ([B, G] group sums, results)."""
    nc = _get_nc()
    in_maps = [
        {"x": importance_scores[i * BL : (i + 1) * BL].reshape(BL * T, D)}
        for i in range(N_CORES)
    ]
    r = run_bass_kernel_spmd(nc, in_maps, core_ids=list(range(N_CORES)), **run_kwargs)

    layout, acc_w = _acc_layout(LANES)

    group_sums = np.zeros((B, G), dtype=np.float64)
    for i, res in enumerate(r.results):
        pe = res["colsum_pe"].astype(np.float64)  # [BL, D]
        group_sums[i * BL : (i + 1) * BL] += pe.reshape(BL, G, GS).sum(axis=-1)
        acc = res["acc"].astype(np.float64).sum(axis=0)  # [acc_w]
        for b, col, width in layout:
            assert width == G, "tiny lane is bench-only"
            group_sums[i * BL + b] += acc[col : col + width]
    return group_sums, r


def kernel(importance_scores, u, W1, b1, ln_g, ln_b, W2, b2, bit_embeddings):
    importance_scores = np.ascontiguousarray(
        np.asarray(importance_scores, dtype=np.float32)
    )

    group_sums, _ = _run_device(importance_scores)
    group_imp = group_sums / (T * GS)

    return _host_tail(
        group_imp,
        np.asarray(u, dtype=np.float64),
        np.asarray(W1, dtype=np.float64),
        np.asarray(b1, dtype=np.float64),
        np.asarray(ln_g, dtype=np.float64),
        np.asarray(ln_b, dtype=np.float64),
        np.asarray(W2, dtype=np.float64),
        np.asarray(b2, dtype=np.float64),
        np.asarray(bit_embeddings, dtype=np.float64),
    )
